# revision 31
# baseline (speedup 1.0000x reference)
"""Trainium2 Bass kernel for nn_LowRankElmanCell.

Math (per timestep t, elementwise over (b, d) given low-rank projections):
    wx = einsum('tbd,rd,or->tbo', x, V_x, U_x)         (bulk matmuls)
    wd = einsum('tbd,rd,or->tbo', x, V_d, U_d)
    cand  = tanh(r_h * h + wx_t + b)
    delta = sigmoid(r_delta * h + wd_t + b_delta)
    h'    = (1-delta) * h + delta * cand
    g     = h' + x_t + b_gate
    out_t = h' * g * sigmoid(g)

Kernel strategy (8 NeuronCores, data-parallel over B, B_local = 2):
  - Substitute k = (h+1)/2 and tanh(z) = 2*sigmoid(2z)-1.  Then with
      U1 = 4*r_h*k + (2*wx + 2*(b - r_h))      s1 = sigmoid(U1)
      U2 = 2*r_d*k + (wd + b_delta - r_delta)  dl = sigmoid(U2)
    the recurrence is simply  k' = k + dl*(s1 - k), and h = 2k - 1.
    One batched sigmoid per step handles both gates (tanh folded away).
  - Bulk phase per 64-step chunk: vx = V_x @ x, then W = U @ vx with the
    bias row folded in via a K=1 accumulating matmul; evicted into the
    scan layout [128 part = d//16, free = (t, q, b, d%16)].
  - Scan phase: 4 small DVE ops + 1 ACT sigmoid per step.
  - Output gate phase (bulk, parallel over t): h = 2k-1 (ACT copy),
    g = h + bg + x (DVE), s = sigmoid(g) (ACT), out = h*(g*s) (GPSIMD).
  - Only activation table set used is sigmoid_and_others (Sigmoid/Copy),
    so there are no mid-kernel activation-table swaps.

The d index maps to (p, j) with d = p*16 + j so that DMA transfers move
64 contiguous bytes per partition per (t, b) — efficient on both sides.
"""

import numpy as np

DIM = 2048
RANK = 128
B = 16
NCORES = 8
BL = B // NCORES  # 2
NJ = 16           # d = p*16 + j
NP = 128

_prog_cache = {}


def _np_reference(x, h0, U_x, V_x, U_d, V_d, r_h, r_delta, b, b_delta, b_gate):
    """Plain numpy fallback (only used if the diagonal vectors are not
    uniform, which the device fast path assumes for r_h/r_delta/b_gate)."""
    T = x.shape[0]
    vx = np.einsum('tbd,rd->tbr', x, V_x)
    wx = np.einsum('tbr,or->tbo', vx, U_x)
    vd = np.einsum('tbd,rd->tbr', x, V_d)
    wd = np.einsum('tbr,or->tbo', vd, U_d)
    h = h0.astype(np.float64)
    outs = np.empty_like(x)
    hs = np.empty_like(x)
    for t in range(T):
        cand = np.tanh(r_h * h + wx[t] + b)
        delta = 1.0 / (1.0 + np.exp(-(wd[t] + r_delta * h + b_delta)))
        h = (1.0 - delta) * h + delta * cand
        g = h + x[t] + b_gate
        outs[t] = (h * (g / (1.0 + np.exp(-g)))).astype(np.float32)
        hs[t] = h.astype(np.float32)
    hfull = np.concatenate([h0[None], hs], axis=0)
    return outs, hfull


def _build_program(T, TC, c1, c2, bg):
    """Build the single-core SPMD Bass program (same program on all cores)."""
    import concourse.bacc as bacc
    import concourse.mybir as mybir
    import concourse.tile as tile

    f32 = mybir.dt.float32
    f32r = mybir.dt.float32r
    AF = mybir.ActivationFunctionType
    OP = mybir.AluOpType
    NCH = T // TC
    ROWS = TC * BL  # matmul moving-dim rows per chunk

    nc = bacc.Bacc()

    x_d = nc.declare_dram_parameter("x", [T, BL, DIM], f32r, isOutput=False)
    uxt_d = nc.declare_dram_parameter("UxT2", [RANK, DIM], f32r, isOutput=False)
    udt_d = nc.declare_dram_parameter("UdT", [RANK, DIM], f32r, isOutput=False)
    vxt_d = nc.declare_dram_parameter("VxT", [DIM, RANK], f32r, isOutput=False)
    vdt_d = nc.declare_dram_parameter("VdT", [DIM, RANK], f32r, isOutput=False)
    bx_d = nc.declare_dram_parameter("bx", [1, DIM], f32r, isOutput=False)
    bd_d = nc.declare_dram_parameter("bd", [1, DIM], f32r, isOutput=False)
    k0_d = nc.declare_dram_parameter("k0", [NP, BL, NJ], f32, isOutput=False)
    out_d = nc.declare_dram_parameter("out", [T, BL, DIM], f32, isOutput=True)
    hs_d = nc.declare_dram_parameter("hs", [T, BL, DIM], f32, isOutput=True)

    # scan-layout views of the DRAM tensors: [p, t, b, j], d = p*16 + j
    x_r = x_d.ap().rearrange("t b (p j) -> p t b j", j=NJ)
    out_r = out_d.ap().rearrange("t b (p j) -> p t b j", j=NJ)
    hs_r = hs_d.ap().rearrange("t b (p j) -> p t b j", j=NJ)

    with tile.TileContext(nc) as tc:
        with tc.tile_pool(name="singles", bufs=1) as singles, \
             tc.tile_pool(name="xc", bufs=2) as xcp, \
             tc.tile_pool(name="wc", bufs=2) as wcp, \
             tc.tile_pool(name="kc", bufs=2) as kcp, \
             tc.tile_pool(name="vs", bufs=2) as vsp, \
             tc.tile_pool(name="bulk", bufs=2) as bulkp, \
             tc.tile_pool(name="scan", bufs=4) as scanp, \
             tc.tile_pool(name="pv", bufs=2, space="PSUM") as pvp, \
             tc.tile_pool(name="pw", bufs=4, space="PSUM") as pwp, \
             tc.tile_pool(name="pd", bufs=1, space="PSUM") as pdp:

            # ---- load parameters (once) ----
            uxtS = singles.tile([RANK, DIM], f32r)
            udtS = singles.tile([RANK, DIM], f32r)
            nc.sync.dma_start(out=uxtS, in_=uxt_d.ap())
            nc.sync.dma_start(out=udtS, in_=udt_d.ap())
            # V^T tiles: [p, j, r] so lhsT for K-chunk j is vxtS[:, j, :]
            vxtS = singles.tile([NP, NJ, RANK], f32r)
            vdtS = singles.tile([NP, NJ, RANK], f32r)
            nc.sync.dma_start(out=vxtS, in_=vxt_d.ap().rearrange("(p j) r -> p j r", j=NJ))
            nc.sync.dma_start(out=vdtS, in_=vdt_d.ap().rearrange("(p j) r -> p j r", j=NJ))
            bxS = singles.tile([1, DIM], f32r)
            bdS = singles.tile([1, DIM], f32r)
            nc.sync.dma_start(out=bxS, in_=bx_d.ap())
            nc.sync.dma_start(out=bdS, in_=bd_d.ap())
            k0S = singles.tile([NP, BL, NJ], f32)
            nc.sync.dma_start(out=k0S, in_=k0_d.ap())
            onesF = singles.tile([1, TC, BL], f32)
            nc.vector.memset(onesF, 1.0)
            # fp32r operands must come from a rounding producer; Memset
            # cannot write fp32r, so round via a DVE copy.
            onesS = singles.tile([1, TC, BL], f32r)
            nc.vector.tensor_copy(onesS, onesF)

            uxt_r = uxtS.rearrange("r (p j) -> r p j", j=NJ)
            udt_r = udtS.rearrange("r (p j) -> r p j", j=NJ)
            bx_r = bxS.rearrange("o (p j) -> o p j", j=NJ)
            bd_r = bdS.rearrange("o (p j) -> o p j", j=NJ)

            # Absorb each parameter-DMA completion into its own throwaway
            # 1x1 matmul: hardware Matmult/LDWEIGHTS instructions can carry
            # only one sync wait, so no real matmul may wait on >1 DMA queue.
            # Each dummy writes a distinct PSUM column of one long-lived tile
            # so the dummies carry no pool-slot waits of their own.
            pdum = pdp.tile([1, 8 + NCH], f32)
            vdum = singles.tile([1, 2 * NCH + 2], f32)
            gdum = singles.tile([1, NCH + 2], f32)
            ndum = 0
            for par in (vxtS[:, 0], vdtS[:, 0], uxtS, udtS, bxS, bdS):
                nc.tensor.matmul(pdum[:, ndum:ndum + 1],
                                 par[0:1, 0:1].bitcast(f32),
                                 par[0:1, 0:1].bitcast(f32),
                                 start=True, stop=True)
                ndum += 1

            # DVE and GPSIMD absorbers for the k0 DMA wait.
            nc.vector.tensor_copy(vdum[:, 2 * NCH:2 * NCH + 1], k0S[0:1, 0, 0:1])
            nc.gpsimd.tensor_copy(gdum[:, 0:1], k0S[0:1, 0, 0:1])

            m0S = singles.tile([NP, BL, NJ], f32)
            nc.vector.memset(m0S, 0.0)

            km1 = k0S
            km2 = k0S
            mprev = m0S
            for c in range(NCH):
                t0 = c * TC
                # ---- stream x chunk in (scan layout) ----
                xC = xcp.tile([NP, TC, BL, NJ], f32r)
                nc.sync.dma_start(out=xC, in_=x_r[:, t0:t0 + TC])

                # Absorb the xC-DMA wait into a throwaway 1x1 matmul so the
                # real matmuls below never carry more than one sync wait.
                nc.tensor.matmul(pdum[:, ndum:ndum + 1],
                                 xC[0:1, 0:1, 0, 0].bitcast(f32),
                                 xC[0:1, 0:1, 0, 0].bitcast(f32),
                                 start=True, stop=True)
                ndum += 1

                # ---- low-rank projections for this chunk ----
                WC = wcp.tile([NP, TC, 2, BL, NJ], f32)
                for q in range(2):
                    vtS = vsp.tile([RANK, TC, BL], f32r, tag="vts")
                    pv = pvp.tile([RANK, TC, BL], f32)
                    vt_lhs = vxtS if q == 0 else vdtS
                    # float32r: single-pass fp32 matmul (~3x faster than the
                    # 4-pass full-precision fp32 mode), bit-compatible input.
                    for j in range(NJ):
                        nc.tensor.matmul(pv, vt_lhs[:, j],
                                         xC[:, :, :, j],
                                         start=(j == 0), stop=(j == NJ - 1))
                    nc.scalar.copy(vtS, pv)
                    u_r = uxt_r if q == 0 else udt_r
                    b_r = bx_r if q == 0 else bd_r
                    for j in range(NJ):
                        pw = pwp.tile([NP, TC, BL], f32)
                        nc.tensor.matmul(pw, u_r[:, :, j],
                                         vtS, start=True, stop=False)
                        nc.tensor.matmul(pw, b_r[:, :, j],
                                         onesS, start=False, stop=True)
                        # PSUM->SBUF evictions on ACT: DVE is saturated by the
                        # scan chain and GPSIMD cannot read PSUM.
                        nc.scalar.copy(WC[:, :, q, :, j], pw)

                # ---- sequential scan over the chunk ----
                # Chain-shortened recurrence: with host-prescaled
                # W~x = (2wx + 2(b-rh))/c1 and W~d = (wd + bd - rd)/c2,
                #   s1_t = sigmoid(c1 * (k_{t-1} + W~x_t))
                #   dl_t = sigmoid(c2 * (k_{t-1} + W~d_t))
                #   m_t  = dl_t * (s1_t - k_{t-1});   k_t = k_{t-1} + m_t
                # and since k_{t-1} = k_{t-2} + m_{t-1}, the sigmoid input is
                #   U~_t = P~_t + m_{t-1},  P~_t = k_{t-2} + W~_t
                # P~ depends on k two steps back, so GPSIMD computes it off
                # the critical chain; the chain is m -> U~ -> sigma -> v -> m.
                kC = kcp.tile([NP, TC, BL, NJ], f32)
                for t in range(TC):
                    Ut = scanp.tile([NP, 2, BL, NJ], f32, name="Ut", tag="Ut")
                    nc.vector.tensor_add(
                        Ut, km1.unsqueeze(1).broadcast_to([NP, 2, BL, NJ]),
                        WC[:, t])
                    St = scanp.tile([NP, 2, BL, NJ], f32, name="St", tag="St")
                    nc.scalar.activation(out=St[:, 0], in_=Ut[:, 0],
                                         func=AF.Sigmoid, scale=c1)
                    nc.scalar.activation(out=St[:, 1], in_=Ut[:, 1],
                                         func=AF.Sigmoid, scale=c2)
                    v = scanp.tile([NP, BL, NJ], f32, name="v", tag="v")
                    nc.vector.tensor_sub(v, St[:, 0], km1)
                    m = scanp.tile([NP, BL, NJ], f32, name="m", tag="m")
                    nc.vector.tensor_mul(m, St[:, 1], v)
                    nc.vector.tensor_add(kC[:, t], km1, m)
                    km1 = kC[:, t]

                # ---- bulk output gate, in [128,512]-sized pieces (8 steps
                # each) streamed straight out to HBM so no full-chunk h/out
                # tile is ever resident and no instruction blocks an engine
                # for long ----
                TP = 512 // (BL * NJ)  # timesteps per piece
                for tt in range(0, TC, TP):
                    kP = kC[:, tt:tt + TP]
                    xP = xC[:, tt:tt + TP]
                    hP = bulkp.tile([NP, TP, BL, NJ], f32, tag="hP")
                    nc.scalar.activation(out=hP, in_=kP, func=AF.Copy,
                                         bias=-1.0, scale=2.0)
                    gP = bulkp.tile([NP, TP, BL, NJ], f32, tag="gP")
                    if float(bg) == 0.0:
                        # Pool has no scalar_tensor_tensor; with bg == 0 a
                        # plain add suffices (bg != 0 falls back to DVE).
                        nc.gpsimd.tensor_add(gP, hP, xP.bitcast(f32))
                    else:
                        nc.vector.scalar_tensor_tensor(
                            out=gP, in0=hP, scalar=float(bg),
                            in1=xP.bitcast(f32), op0=OP.add, op1=OP.add)
                    sP = bulkp.tile([NP, TP, BL, NJ], f32, tag="sP")
                    nc.scalar.activation(out=sP, in_=gP, func=AF.Sigmoid)
                    t1P = bulkp.tile([NP, TP, BL, NJ], f32, tag="t1P")
                    nc.gpsimd.tensor_mul(t1P, gP, sP)
                    oP = bulkp.tile([NP, TP, BL, NJ], f32, tag="oP")
                    nc.gpsimd.tensor_mul(oP, hP, t1P)
                    nc.sync.dma_start(out=hs_r[:, t0 + tt:t0 + tt + TP], in_=hP)
                    nc.sync.dma_start(out=out_r[:, t0 + tt:t0 + tt + TP], in_=oP)

    # bacc pipeline: register alloc, nop fusion, and crucially
    # generate_event_semaphores() — splits multi-wait sync lists to satisfy
    # the TRN2 one-wait-per-instruction hardware constraint.
    nc.compile()
    return nc


def _get_program(T, TC, c1, c2, bg):
    key = (T, TC, c1, c2, bg)
    if key not in _prog_cache:
        _prog_cache[key] = _build_program(T, TC, c1, c2, bg)
    return _prog_cache[key]


def run_device(x, h0, U_x, V_x, U_d, V_d, r_h, r_delta, b, b_delta, b_gate,
               TC=128, trace=False):
    """Shard over B, run the SPMD program, gather.  Returns (out, h[, res])."""
    from concourse.bass_utils import run_bass_kernel_spmd

    T = x.shape[0]
    c1 = float(4.0 * r_h[0])
    c2 = float(2.0 * r_delta[0])
    bg = float(b_gate[0])

    nc = _get_program(T, TC, c1, c2, bg)

    # Weights and biases are pre-scaled so the device sigmoid inputs are
    # k + W~ with the c1/c2 factors applied via the ACT scale immediate.
    UxT2 = np.ascontiguousarray((2.0 / c1) * U_x.T).astype(np.float32)
    UdT = np.ascontiguousarray((1.0 / c2) * U_d.T).astype(np.float32)
    VxT = np.ascontiguousarray(V_x.T).astype(np.float32)
    VdT = np.ascontiguousarray(V_d.T).astype(np.float32)
    bx = ((2.0 / c1) * (b - r_h)).astype(np.float32).reshape(1, DIM)
    bd = ((b_delta - r_delta) / c2).astype(np.float32).reshape(1, DIM)
    k0 = ((h0 + 1.0) * 0.5).astype(np.float32)  # [B, DIM]

    in_maps = []
    for i in range(NCORES):
        bs = slice(BL * i, BL * (i + 1))
        k0i = k0[bs].reshape(BL, NP, NJ).transpose(1, 0, 2)  # [p, b, j]
        in_maps.append({
            "x": np.ascontiguousarray(x[:, bs, :]),
            "UxT2": UxT2, "UdT": UdT, "VxT": VxT, "VdT": VdT,
            "bx": bx, "bd": bd,
            "k0": np.ascontiguousarray(k0i),
        })

    res = run_bass_kernel_spmd(nc, in_maps, list(range(NCORES)), trace=trace)

    out = np.empty((T, B, DIM), np.float32)
    hs = np.empty((T, B, DIM), np.float32)
    for i in range(NCORES):
        bs = slice(BL * i, BL * (i + 1))
        out[:, bs, :] = res.results[i]["out"]
        hs[:, bs, :] = res.results[i]["hs"]
    h = np.concatenate([h0[None].astype(np.float32), hs], axis=0)
    if trace:
        return out, h, res
    return out, h


def kernel(x, h0, U_x, V_x, U_d, V_d, r_h, r_delta, b, b_delta, b_gate):
    x = np.asarray(x, np.float32)
    h0 = np.asarray(h0, np.float32)
    args = [np.asarray(a, np.float32) for a in
            (U_x, V_x, U_d, V_d, r_h, r_delta, b, b_delta, b_gate)]
    U_x, V_x, U_d, V_d, r_h, r_delta, b, b_delta, b_gate = args

    uniform = all(np.all(v == v.flat[0]) for v in (r_h, r_delta, b_gate))
    if not uniform or x.shape != (1024, B, DIM):
        return _np_reference(x, h0, U_x, V_x, U_d, V_d, r_h, r_delta,
                             b, b_delta, b_gate)

    out, h = run_device(x, h0, U_x, V_x, U_d, V_d, r_h, r_delta,
                        b, b_delta, b_gate)
    return out, h


# revision 33
# speedup vs baseline: 1.0643x; 1.0643x over previous
"""Trainium2 Bass kernel for nn_LowRankElmanCell.

Math (per timestep t, elementwise over (b, d) given low-rank projections):
    wx = einsum('tbd,rd,or->tbo', x, V_x, U_x)         (bulk matmuls)
    wd = einsum('tbd,rd,or->tbo', x, V_d, U_d)
    cand  = tanh(r_h * h + wx_t + b)
    delta = sigmoid(r_delta * h + wd_t + b_delta)
    h'    = (1-delta) * h + delta * cand
    g     = h' + x_t + b_gate
    out_t = h' * g * sigmoid(g)

Kernel strategy (8 NeuronCores, data-parallel over B, B_local = 2):
  - Substitute k = (h+1)/2 and tanh(z) = 2*sigmoid(2z)-1.  With c1 = 4*r_h,
    c2 = 2*r_delta and host-prescaled projections
      W~x_t = (2*wx_t + 2*(b - r_h)) / c1,  W~d_t = (wd_t + b_d - r_d) / c2
    the per-step recurrence is
      s1 = sigmoid(c1 * (k + W~x_t))   [ACT, scale immediate]
      dl = sigmoid(c2 * (k + W~d_t))   [ACT, scale immediate]
      k' = k + dl*(s1 - k),   h = 2k - 1
  - Bulk phase per 128-step chunk (float32r single-pass matmuls): vx = V@x
    (K=d accumulation), then W~ = U@vx with the bias row folded in via a
    K=1 accumulating matmul; ACT-evicted into the scan layout
    [128 part = d//16, free = (t, q, b, d%16)].
  - Scan phase per step (critical chain, all-DVE + ACT):
    U = bcast(k) + W~_t (one DVE tt via stride-0 broadcast), two sigmoids,
    v = s1 - k, m = dl*v, k' = k + m (DVE).  ~1.4us/step model-bound by
    per-instruction fixed costs + the sigmoid round-trip.
  - Output gate (bulk, parallel over t, [128,512] pieces streamed to HBM):
    h = 2k-1 (ACT copy scale/bias), g = h + x (+bg) (Pool), s = sigmoid(g)
    (ACT), t1 = g*s (Pool), out = h*t1 (Pool).
  - Only activation table set used is sigmoid_and_others (Sigmoid/Copy),
    so there are no mid-kernel activation-table swaps.
  - Dummy 1x1 matmuls / tiny copies absorb DMA-queue waits because TRN2
    engine instructions carry at most one semaphore wait (the rest would
    become event-semaphore instructions serializing the sequencers).

The d index maps to (p, j) with d = p*16 + j so that DMA transfers move
64 contiguous bytes per partition per (t, b) — efficient on both sides.
"""

import numpy as np

DIM = 2048
RANK = 128
B = 16
NCORES = 8
BL = B // NCORES  # 2
NJ = 16           # d = p*16 + j
NP = 128

_prog_cache = {}


def _np_reference(x, h0, U_x, V_x, U_d, V_d, r_h, r_delta, b, b_delta, b_gate):
    """Plain numpy fallback (only used if the diagonal vectors are not
    uniform, which the device fast path assumes for r_h/r_delta/b_gate)."""
    T = x.shape[0]
    vx = np.einsum('tbd,rd->tbr', x, V_x)
    wx = np.einsum('tbr,or->tbo', vx, U_x)
    vd = np.einsum('tbd,rd->tbr', x, V_d)
    wd = np.einsum('tbr,or->tbo', vd, U_d)
    h = h0.astype(np.float64)
    outs = np.empty_like(x)
    hs = np.empty_like(x)
    for t in range(T):
        cand = np.tanh(r_h * h + wx[t] + b)
        delta = 1.0 / (1.0 + np.exp(-(wd[t] + r_delta * h + b_delta)))
        h = (1.0 - delta) * h + delta * cand
        g = h + x[t] + b_gate
        outs[t] = (h * (g / (1.0 + np.exp(-g)))).astype(np.float32)
        hs[t] = h.astype(np.float32)
    hfull = np.concatenate([h0[None], hs], axis=0)
    return outs, hfull


def _build_program(T, TC, c1, c2, bg):
    """Build the single-core SPMD Bass program (same program on all cores)."""
    import concourse.bacc as bacc
    import concourse.mybir as mybir
    import concourse.tile as tile

    f32 = mybir.dt.float32
    f32r = mybir.dt.float32r
    AF = mybir.ActivationFunctionType
    OP = mybir.AluOpType
    NCH = T // TC
    ROWS = TC * BL  # matmul moving-dim rows per chunk

    nc = bacc.Bacc()

    x_d = nc.declare_dram_parameter("x", [T, BL, DIM], f32r, isOutput=False)
    uxt_d = nc.declare_dram_parameter("UxT2", [RANK, DIM], f32r, isOutput=False)
    udt_d = nc.declare_dram_parameter("UdT", [RANK, DIM], f32r, isOutput=False)
    vxt_d = nc.declare_dram_parameter("VxT", [DIM, RANK], f32r, isOutput=False)
    vdt_d = nc.declare_dram_parameter("VdT", [DIM, RANK], f32r, isOutput=False)
    bx_d = nc.declare_dram_parameter("bx", [1, DIM], f32r, isOutput=False)
    bd_d = nc.declare_dram_parameter("bd", [1, DIM], f32r, isOutput=False)
    k0_d = nc.declare_dram_parameter("k0", [NP, BL, NJ], f32, isOutput=False)
    out_d = nc.declare_dram_parameter("out", [T, BL, DIM], f32, isOutput=True)
    hs_d = nc.declare_dram_parameter("hs", [T, BL, DIM], f32, isOutput=True)

    # scan-layout views of the DRAM tensors: [p, t, b, j], d = p*16 + j
    x_r = x_d.ap().rearrange("t b (p j) -> p t b j", j=NJ)
    out_r = out_d.ap().rearrange("t b (p j) -> p t b j", j=NJ)
    hs_r = hs_d.ap().rearrange("t b (p j) -> p t b j", j=NJ)

    with tile.TileContext(nc) as tc:
        with tc.tile_pool(name="singles", bufs=1) as singles, \
             tc.tile_pool(name="xc", bufs=2) as xcp, \
             tc.tile_pool(name="wc", bufs=2) as wcp, \
             tc.tile_pool(name="kc", bufs=2) as kcp, \
             tc.tile_pool(name="vs", bufs=2) as vsp, \
             tc.tile_pool(name="bulk", bufs=2) as bulkp, \
             tc.tile_pool(name="scan", bufs=4) as scanp, \
             tc.tile_pool(name="pv", bufs=2, space="PSUM") as pvp, \
             tc.tile_pool(name="pw", bufs=4, space="PSUM") as pwp, \
             tc.tile_pool(name="pd", bufs=1, space="PSUM") as pdp:

            # ---- load parameters (once) ----
            uxtS = singles.tile([RANK, DIM], f32r)
            udtS = singles.tile([RANK, DIM], f32r)
            nc.sync.dma_start(out=uxtS, in_=uxt_d.ap())
            nc.sync.dma_start(out=udtS, in_=udt_d.ap())
            # V^T tiles: [p, j, r] so lhsT for K-chunk j is vxtS[:, j, :]
            vxtS = singles.tile([NP, NJ, RANK], f32r)
            vdtS = singles.tile([NP, NJ, RANK], f32r)
            nc.sync.dma_start(out=vxtS, in_=vxt_d.ap().rearrange("(p j) r -> p j r", j=NJ))
            nc.sync.dma_start(out=vdtS, in_=vdt_d.ap().rearrange("(p j) r -> p j r", j=NJ))
            bxS = singles.tile([1, DIM], f32r)
            bdS = singles.tile([1, DIM], f32r)
            nc.sync.dma_start(out=bxS, in_=bx_d.ap())
            nc.sync.dma_start(out=bdS, in_=bd_d.ap())
            k0S = singles.tile([NP, BL, NJ], f32)
            nc.sync.dma_start(out=k0S, in_=k0_d.ap())
            onesF = singles.tile([1, TC, BL], f32)
            nc.vector.memset(onesF, 1.0)
            # fp32r operands must come from a rounding producer; Memset
            # cannot write fp32r, so round via a DVE copy.
            onesS = singles.tile([1, TC, BL], f32r)
            nc.vector.tensor_copy(onesS, onesF)

            uxt_r = uxtS.rearrange("r (p j) -> r p j", j=NJ)
            udt_r = udtS.rearrange("r (p j) -> r p j", j=NJ)
            bx_r = bxS.rearrange("o (p j) -> o p j", j=NJ)
            bd_r = bdS.rearrange("o (p j) -> o p j", j=NJ)

            # Absorb each parameter-DMA completion into its own throwaway
            # 1x1 matmul: hardware Matmult/LDWEIGHTS instructions can carry
            # only one sync wait, so no real matmul may wait on >1 DMA queue.
            # Each dummy writes a distinct PSUM column of one long-lived tile
            # so the dummies carry no pool-slot waits of their own.
            pdum = pdp.tile([1, 8 + NCH], f32)
            vdum = singles.tile([1, 2 * NCH + 2], f32)
            gdum = singles.tile([1, NCH + 2], f32)
            ndum = 0
            for par in (vxtS[:, 0], vdtS[:, 0], uxtS, udtS, bxS, bdS):
                nc.tensor.matmul(pdum[:, ndum:ndum + 1],
                                 par[0:1, 0:1].bitcast(f32),
                                 par[0:1, 0:1].bitcast(f32),
                                 start=True, stop=True)
                ndum += 1

            # DVE and GPSIMD absorbers for the k0 DMA wait.
            nc.vector.tensor_copy(vdum[:, 2 * NCH:2 * NCH + 1], k0S[0:1, 0, 0:1])
            nc.gpsimd.tensor_copy(gdum[:, 0:1], k0S[0:1, 0, 0:1])

            m0S = singles.tile([NP, BL, NJ], f32)
            nc.vector.memset(m0S, 0.0)

            km1 = k0S
            km2 = k0S
            mprev = m0S
            for c in range(NCH):
                t0 = c * TC
                # ---- stream x chunk in (scan layout) ----
                xC = xcp.tile([NP, TC, BL, NJ], f32r)
                nc.sync.dma_start(out=xC, in_=x_r[:, t0:t0 + TC])

                # Absorb the xC-DMA wait into a throwaway 1x1 matmul so the
                # real matmuls below never carry more than one sync wait.
                nc.tensor.matmul(pdum[:, ndum:ndum + 1],
                                 xC[0:1, 0:1, 0, 0].bitcast(f32),
                                 xC[0:1, 0:1, 0, 0].bitcast(f32),
                                 start=True, stop=True)
                ndum += 1

                # ---- low-rank projections for this chunk ----
                WC = wcp.tile([NP, TC, 2, BL, NJ], f32)
                for q in range(2):
                    vtS = vsp.tile([RANK, TC, BL], f32r, tag="vts")
                    pv = pvp.tile([RANK, TC, BL], f32)
                    vt_lhs = vxtS if q == 0 else vdtS
                    # float32r: single-pass fp32 matmul (~3x faster than the
                    # 4-pass full-precision fp32 mode), bit-compatible input.
                    for j in range(NJ):
                        nc.tensor.matmul(pv, vt_lhs[:, j],
                                         xC[:, :, :, j],
                                         start=(j == 0), stop=(j == NJ - 1))
                    nc.scalar.copy(vtS, pv)
                    u_r = uxt_r if q == 0 else udt_r
                    b_r = bx_r if q == 0 else bd_r
                    for j in range(NJ):
                        pw = pwp.tile([NP, TC, BL], f32)
                        nc.tensor.matmul(pw, u_r[:, :, j],
                                         vtS, start=True, stop=False)
                        nc.tensor.matmul(pw, b_r[:, :, j],
                                         onesS, start=False, stop=True)
                        # PSUM->SBUF evictions on ACT: DVE is saturated by the
                        # scan chain and GPSIMD cannot read PSUM.
                        nc.scalar.copy(WC[:, :, q, :, j], pw)

                # ---- sequential scan over the chunk ----
                # Chain-shortened recurrence: with host-prescaled
                # W~x = (2wx + 2(b-rh))/c1 and W~d = (wd + bd - rd)/c2,
                #   s1_t = sigmoid(c1 * (k_{t-1} + W~x_t))
                #   dl_t = sigmoid(c2 * (k_{t-1} + W~d_t))
                #   m_t  = dl_t * (s1_t - k_{t-1});   k_t = k_{t-1} + m_t
                # and since k_{t-1} = k_{t-2} + m_{t-1}, the sigmoid input is
                #   U~_t = P~_t + m_{t-1},  P~_t = k_{t-2} + W~_t
                # P~ depends on k two steps back, so GPSIMD computes it off
                # the critical chain; the chain is m -> U~ -> sigma -> v -> m.
                kC = kcp.tile([NP, TC, BL, NJ], f32)
                for t in range(TC):
                    Pt = scanp.tile([NP, 2, BL, NJ], f32, name="Pt", tag="Pt")
                    nc.vector.tensor_add(
                        Pt, km2.unsqueeze(1).broadcast_to([NP, 2, BL, NJ]),
                        WC[:, t])
                    Ut = scanp.tile([NP, 2, BL, NJ], f32, name="Ut", tag="Ut")
                    nc.vector.tensor_add(
                        Ut, mprev.unsqueeze(1).broadcast_to([NP, 2, BL, NJ]),
                        Pt)
                    St = scanp.tile([NP, 2, BL, NJ], f32, name="St", tag="St")
                    nc.scalar.activation(out=St[:, 0], in_=Ut[:, 0],
                                         func=AF.Sigmoid, scale=c1)
                    nc.scalar.activation(out=St[:, 1], in_=Ut[:, 1],
                                         func=AF.Sigmoid, scale=c2)
                    v = scanp.tile([NP, BL, NJ], f32, name="v", tag="v")
                    nc.vector.tensor_sub(v, St[:, 0], km1)
                    m = scanp.tile([NP, BL, NJ], f32, name="m", tag="m")
                    nc.vector.tensor_mul(m, St[:, 1], v)
                    nc.vector.tensor_add(kC[:, t], km1, m)
                    km2 = km1
                    km1 = kC[:, t]
                    mprev = m

                # ---- bulk output gate, in [128,512]-sized pieces (8 steps
                # each) streamed straight out to HBM so no full-chunk h/out
                # tile is ever resident and no instruction blocks an engine
                # for long ----
                TP = 512 // (BL * NJ)  # timesteps per piece
                for tt in range(0, TC, TP):
                    kP = kC[:, tt:tt + TP]
                    xP = xC[:, tt:tt + TP]
                    hP = bulkp.tile([NP, TP, BL, NJ], f32, tag="hP")
                    nc.scalar.activation(out=hP, in_=kP, func=AF.Copy,
                                         bias=-1.0, scale=2.0)
                    gP = bulkp.tile([NP, TP, BL, NJ], f32, tag="gP")
                    if float(bg) == 0.0:
                        # Pool has no scalar_tensor_tensor; with bg == 0 a
                        # plain add suffices (bg != 0 falls back to DVE).
                        nc.gpsimd.tensor_add(gP, hP, xP.bitcast(f32))
                    else:
                        nc.vector.scalar_tensor_tensor(
                            out=gP, in0=hP, scalar=float(bg),
                            in1=xP.bitcast(f32), op0=OP.add, op1=OP.add)
                    sP = bulkp.tile([NP, TP, BL, NJ], f32, tag="sP")
                    nc.scalar.activation(out=sP, in_=gP, func=AF.Sigmoid)
                    t1P = bulkp.tile([NP, TP, BL, NJ], f32, tag="t1P")
                    nc.gpsimd.tensor_mul(t1P, gP, sP)
                    oP = bulkp.tile([NP, TP, BL, NJ], f32, tag="oP")
                    nc.gpsimd.tensor_mul(oP, hP, t1P)
                    nc.sync.dma_start(out=hs_r[:, t0 + tt:t0 + tt + TP], in_=hP)
                    nc.sync.dma_start(out=out_r[:, t0 + tt:t0 + tt + TP], in_=oP)

    # bacc pipeline: register alloc, nop fusion, and crucially
    # generate_event_semaphores() — splits multi-wait sync lists to satisfy
    # the TRN2 one-wait-per-instruction hardware constraint.
    nc.compile()
    return nc


def _get_program(T, TC, c1, c2, bg):
    key = (T, TC, c1, c2, bg)
    if key not in _prog_cache:
        _prog_cache[key] = _build_program(T, TC, c1, c2, bg)
    return _prog_cache[key]


def run_device(x, h0, U_x, V_x, U_d, V_d, r_h, r_delta, b, b_delta, b_gate,
               TC=128, trace=False):
    """Shard over B, run the SPMD program, gather.  Returns (out, h[, res])."""
    from concourse.bass_utils import run_bass_kernel_spmd

    T = x.shape[0]
    c1 = float(4.0 * r_h[0])
    c2 = float(2.0 * r_delta[0])
    bg = float(b_gate[0])

    nc = _get_program(T, TC, c1, c2, bg)

    # Weights and biases are pre-scaled so the device sigmoid inputs are
    # k + W~ with the c1/c2 factors applied via the ACT scale immediate.
    UxT2 = np.ascontiguousarray((2.0 / c1) * U_x.T).astype(np.float32)
    UdT = np.ascontiguousarray((1.0 / c2) * U_d.T).astype(np.float32)
    VxT = np.ascontiguousarray(V_x.T).astype(np.float32)
    VdT = np.ascontiguousarray(V_d.T).astype(np.float32)
    bx = ((2.0 / c1) * (b - r_h)).astype(np.float32).reshape(1, DIM)
    bd = ((b_delta - r_delta) / c2).astype(np.float32).reshape(1, DIM)
    k0 = ((h0 + 1.0) * 0.5).astype(np.float32)  # [B, DIM]

    in_maps = []
    for i in range(NCORES):
        bs = slice(BL * i, BL * (i + 1))
        k0i = k0[bs].reshape(BL, NP, NJ).transpose(1, 0, 2)  # [p, b, j]
        in_maps.append({
            "x": np.ascontiguousarray(x[:, bs, :]),
            "UxT2": UxT2, "UdT": UdT, "VxT": VxT, "VdT": VdT,
            "bx": bx, "bd": bd,
            "k0": np.ascontiguousarray(k0i),
        })

    res = run_bass_kernel_spmd(nc, in_maps, list(range(NCORES)), trace=trace)

    out = np.empty((T, B, DIM), np.float32)
    hs = np.empty((T, B, DIM), np.float32)
    for i in range(NCORES):
        bs = slice(BL * i, BL * (i + 1))
        out[:, bs, :] = res.results[i]["out"]
        hs[:, bs, :] = res.results[i]["hs"]
    h = np.concatenate([h0[None].astype(np.float32), hs], axis=0)
    if trace:
        return out, h, res
    return out, h


def kernel(x, h0, U_x, V_x, U_d, V_d, r_h, r_delta, b, b_delta, b_gate):
    x = np.asarray(x, np.float32)
    h0 = np.asarray(h0, np.float32)
    args = [np.asarray(a, np.float32) for a in
            (U_x, V_x, U_d, V_d, r_h, r_delta, b, b_delta, b_gate)]
    U_x, V_x, U_d, V_d, r_h, r_delta, b, b_delta, b_gate = args

    uniform = all(np.all(v == v.flat[0]) for v in (r_h, r_delta, b_gate))
    if not uniform or x.shape != (1024, B, DIM):
        return _np_reference(x, h0, U_x, V_x, U_d, V_d, r_h, r_delta,
                             b, b_delta, b_gate)

    out, h = run_device(x, h0, U_x, V_x, U_d, V_d, r_h, r_delta,
                        b, b_delta, b_gate)
    return out, h
